# revision 13
# baseline (speedup 1.0000x reference)
"""Trainium2 Bass kernel: causal attention with RoPE (B=2, T=2048, H=16, hd=64).

Sharding: 8 cores = 2 batches x 4 head-groups (4 heads each).
Per core: qkv projection (its 768 weight columns), RoPE, causal attention
(head-parallel), partial out-projection; ReduceScatter over each 4-core
group sums the out-projection partials. Host only slices/permutes inputs
and concatenates/permutes outputs.

Layout choices (all transposes are host-side or folded into weights):
  - x is fed transposed: xT (D=1024, T=2048); q,k computed d-major
    (d on partitions), v computed T-major with an appended ones column.
  - Wq/Wk columns host-permuted so each head's q/k rows come out as
    [32 even dims, 32 odd dims] -> RoPE is a 64-row half swap (SBUF->SBUF
    DMA) + 3 elementwise ops against host cos/sin tables.
  - Scores computed transposed (S^T: tk on partitions, tq free) so the
    softmax denominator is produced by the ones column of v_aug during
    the ctx matmul; per-tq reciprocal is broadcast via a K=1 matmul.
  - exp uses a constant shift (exact cancellation in softmax).
"""
import sys
import numpy as np

for _p in ("/opt/trn_rl_repo", "/opt/pypackages"):
    if _p not in sys.path:
        sys.path.append(_p)

import concourse.bass as bass
import concourse.tile as tile
from concourse import bacc, mybir
from concourse.bass_utils import run_bass_kernel_spmd

B, T, NH, HD = 2, 2048, 16, 64
D = NH * HD                      # 1024
HPC = 4                          # heads per core
DPC = HPC * HD                   # 256
N_CORES = 8
GROUPS = [[0, 1, 2, 3], [4, 5, 6, 7]]
ROPE_BASE = 10000.0
SHIFT = 12.0                     # constant pre-exp shift; cancels exactly
TC = 512                         # t chunk (moving free dim)
NTC = T // TC                    # 4
F32 = mybir.dt.float32
F32R = mybir.dt.float32r

MM_DT = F32                      # matmul input dtype

_BUILD_CACHE = {}


def _mm(ap):
    """View an fp32 AP with the matmul input dtype."""
    return ap.bitcast(MM_DT) if MM_DT != F32 else ap


def build_nc():
    nc = bacc.Bacc(None, target_bir_lowering=False, num_devices=N_CORES)

    xT = nc.declare_dram_parameter("xT", [D, T], F32, isOutput=False)
    wq = nc.declare_dram_parameter("wq", [D, DPC], F32, isOutput=False)
    wk = nc.declare_dram_parameter("wk", [D, DPC], F32, isOutput=False)
    wv = nc.declare_dram_parameter("wv", [D, DPC], F32, isOutput=False)
    bq = nc.declare_dram_parameter("bq", [DPC], F32, isOutput=False)
    bk = nc.declare_dram_parameter("bk", [DPC], F32, isOutput=False)
    bv = nc.declare_dram_parameter("bv", [DPC], F32, isOutput=False)
    wout = nc.declare_dram_parameter("wout", [DPC, D], F32, isOutput=False)
    bo4 = nc.declare_dram_parameter("bo4", [D], F32, isOutput=False)
    cc = nc.declare_dram_parameter("cc", [128, T], F32, isOutput=False)
    ss = nc.declare_dram_parameter("ss", [128, T], F32, isOutput=False)
    tri = nc.declare_dram_parameter("tri", [128, 128], F32, isOutput=False)

    out_part = nc.declare_dram_parameter("out_part", [DPC, T], F32, isOutput=True)
    k_part = nc.declare_dram_parameter("k_part", [DPC, T], F32, isOutput=True)
    v_part = nc.declare_dram_parameter("v_part", [T, DPC], F32, isOutput=True)

    cc_in = nc.dram_tensor("cc_in", [D, T], F32)
    cc_out = nc.dram_tensor("cc_out", [DPC, T], F32)

    from contextlib import ExitStack
    with tile.TileContext(nc) as tc, ExitStack() as ectx:
        consts = ectx.enter_context(tc.tile_pool(name="consts", bufs=1))
        xpool = ectx.enter_context(tc.tile_pool(name="xpool", bufs=3))
        qkpool = ectx.enter_context(tc.tile_pool(name="qkpool", bufs=1))
        vpool = ectx.enter_context(tc.tile_pool(name="vpool", bufs=1))
        ctxpool = ectx.enter_context(tc.tile_pool(name="ctxpool", bufs=1))
        espool = ectx.enter_context(tc.tile_pool(name="espool", bufs=6))
        swpool = ectx.enter_context(tc.tile_pool(name="swpool", bufs=2))
        smpool = ectx.enter_context(tc.tile_pool(name="smpool", bufs=4))
        opool = ectx.enter_context(tc.tile_pool(name="opool", bufs=3))

        # ---- constants into SBUF ----
        wq_sb = consts.tile([128, 8, DPC], F32)
        nc.sync.dma_start(out=wq_sb, in_=wq.rearrange("(kc kp) m -> kp kc m", kp=128))
        wk_sb = consts.tile([128, 8, DPC], F32)
        nc.sync.dma_start(out=wk_sb, in_=wk.rearrange("(kc kp) m -> kp kc m", kp=128))
        wv_sb = consts.tile([128, 8, DPC], F32)
        nc.sync.dma_start(out=wv_sb, in_=wv.rearrange("(kc kp) m -> kp kc m", kp=128))
        wout_sb = consts.tile([128, 2, D], F32)
        nc.sync.dma_start(out=wout_sb, in_=wout.rearrange("(kc kp) m -> kp kc m", kp=128))
        bq_sb = consts.tile([128, 2], F32)
        nc.sync.dma_start(out=bq_sb, in_=bq.rearrange("(t p) -> p t", p=128))
        bk_sb = consts.tile([128, 2], F32)
        nc.sync.dma_start(out=bk_sb, in_=bk.rearrange("(t p) -> p t", p=128))
        bv_sb = consts.tile([1, DPC], F32)
        nc.sync.dma_start(out=bv_sb, in_=bv[None, :])
        bo_sb = consts.tile([128, 8], F32)
        nc.sync.dma_start(out=bo_sb, in_=bo4.rearrange("(t p) -> p t", p=128))
        cc_sb = consts.tile([128, T], F32)
        nc.sync.dma_start(out=cc_sb, in_=cc[:, :])
        ss_sb = consts.tile([128, T], F32)
        nc.sync.dma_start(out=ss_sb, in_=ss[:, :])
        tri_sb = consts.tile([128, 128], F32)
        nc.sync.dma_start(out=tri_sb, in_=tri[:, :])
        ones_sb = consts.tile([128, 128], F32)
        nc.vector.memset(ones_sb, 1.0)
        shift_sb = consts.tile([128, 1], F32)
        nc.vector.memset(shift_sb, -SHIFT)

        # persistent activations
        q_t = [qkpool.tile([128, T], F32, tag=f"q{i}", name=f"q{i}") for i in range(2)]
        k_t = [qkpool.tile([128, T], F32, tag=f"k{i}", name=f"k{i}") for i in range(2)]
        # v_aug: (p, tk_tile, head, 65); col 64 is the ones column
        v_sb = vpool.tile([128, T // 128, HPC, HD + 1], F32)
        ctx_t = [ctxpool.tile([128, T], F32, tag=f"c{i}", name=f"c{i}") for i in range(2)]

        nc.vector.memset(v_sb[:, :, :, HD:HD + 1], 1.0)

        # ---- phase 1: qkv projection ----
        ps1_cm = tc.tile_pool(name="ps1", bufs=1, space="PSUM")
        ps1 = ps1_cm.__enter__()
        for tc_ in range(NTC):
            ts = bass.ds(tc_ * TC, TC)
            qps = [ps1.tile([128, TC], F32, tag=f"qps{m}", name=f"qps{m}") for m in range(2)]
            kps = [ps1.tile([128, TC], F32, tag=f"kps{m}", name=f"kps{m}") for m in range(2)]
            vps = [ps1.tile([128, DPC], F32, tag=f"vps{s}", name=f"vps{s}")
                   for s in range(4)]  # one PSUM bank per t-subtile
            for kc in range(8):
                xt = xpool.tile([128, TC], F32, tag="x")
                nc.sync.dma_start(out=xt, in_=xT[bass.ds(kc * 128, 128), ts])
                for mt in range(2):
                    ms = bass.ds(mt * 128, 128)
                    nc.tensor.matmul(
                        qps[mt], _mm(wq_sb[:, kc, ms]), _mm(xt),
                        start=(kc == 0), stop=(kc == 7))
                    nc.tensor.matmul(
                        kps[mt], _mm(wk_sb[:, kc, ms]), _mm(xt),
                        start=(kc == 0), stop=(kc == 7))
                for s in range(4):
                    nc.tensor.matmul(
                        vps[s],
                        _mm(xt[:, bass.ds(s * 128, 128)]), _mm(wv_sb[:, kc, :]),
                        start=(kc == 0), stop=False, skip_group_check=True)
            for s in range(4):  # bias row for v (K=1 matmul of ones x bv)
                nc.tensor.matmul(
                    vps[s],
                    _mm(ones_sb[0:1, 0:128]), _mm(bv_sb[0:1, :]),
                    start=False, stop=True, skip_group_check=True)
            for mt in range(2):
                nc.vector.tensor_scalar_add(q_t[mt][:, ts], qps[mt], bq_sb[:, mt:mt + 1])
                nc.vector.tensor_scalar_add(k_t[mt][:, ts], kps[mt], bk_sb[:, mt:mt + 1])
            for s in range(4):
                tt = tc_ * 4 + s
                nc.vector.tensor_copy(
                    out=v_sb[:, tt, :, 0:HD],
                    in_=vps[s].rearrange("p (h d) -> p h d", h=HPC))

        # ---- phase 1.5: RoPE on q,k; write k/v cache ----
        for tl in (q_t[0], q_t[1], k_t[0], k_t[1]):
            sw = swpool.tile([128, T], F32, tag="sw")
            for h2 in range(2):
                b0 = h2 * 64
                nc.sync.dma_start(out=sw[bass.ds(b0, 32), :], in_=tl[bass.ds(b0 + 32, 32), :])
                nc.sync.dma_start(out=sw[bass.ds(b0 + 32, 32), :], in_=tl[bass.ds(b0, 32), :])
            nc.vector.tensor_mul(tl, tl, cc_sb)
            nc.vector.tensor_mul(sw, sw, ss_sb)
            nc.vector.tensor_add(tl, tl, sw)
        for mt in range(2):
            nc.sync.dma_start(out=k_part[bass.ds(mt * 128, 128), :], in_=k_t[mt])
        for h in range(HPC):
            nc.sync.dma_start(
                out=v_part[:, bass.ds(h * HD, HD)].rearrange("(tt p) d -> p tt d", p=128),
                in_=v_sb[:, :, h, 0:HD])

        ps1_cm.__exit__(None, None, None)

        # ---- phase 2: attention ----
        psS = ectx.enter_context(tc.tile_pool(name="psS", bufs=3, space="PSUM"))
        psC = ectx.enter_context(tc.tile_pool(name="psC", bufs=2, space="PSUM"))
        psB = ectx.enter_context(tc.tile_pool(name="psB", bufs=1, space="PSUM"))
        psO = ectx.enter_context(tc.tile_pool(name="psO", bufs=2, space="PSUM"))
        for c in range(NTC):
            cs = bass.ds(c * TC, TC)
            for h in range(HPC):
                kt = k_t[h // 2]
                qt = q_t[h // 2]
                base = (h % 2) * 64
                ntt = 4 * c + 4
                ctxps = psC.tile([HD + 1, TC], F32, tag="ctxps")
                for tt in range(ntt):
                    j = tt - 4 * c
                    sp = psS.tile([128, TC], F32, tag="sps")
                    es = espool.tile([128, TC], F32, tag="es")
                    lhs = kt[bass.ds(base, 64), bass.ds(tt * 128, 128)]
                    if j < 0:  # full tile
                        nc.tensor.matmul(sp, _mm(lhs), _mm(qt[bass.ds(base, 64), cs]),
                                         start=True, stop=True)
                        nc.scalar.activation(es, sp, mybir.ActivationFunctionType.Exp,
                                             bias=shift_sb[:, 0:1], scale=0.125)
                    else:  # diagonal-crossing tile
                        w = TC - j * 128
                        nc.tensor.matmul(
                            sp[:, 0:w], _mm(lhs),
                            _mm(qt[bass.ds(base, 64), bass.ds(c * TC + j * 128, w)]),
                            start=True, stop=True)
                        if j > 0:
                            nc.vector.memset(es[:, 0:j * 128], 0.0)
                        nc.scalar.activation(es[:, bass.ds(j * 128, w)], sp[:, 0:w],
                                             mybir.ActivationFunctionType.Exp,
                                             bias=shift_sb[:, 0:1], scale=0.125)
                        nc.vector.tensor_mul(es[:, bass.ds(j * 128, 128)],
                                             es[:, bass.ds(j * 128, 128)], tri_sb)
                    nc.tensor.matmul(ctxps, _mm(v_sb[:, tt, h, :]), _mm(es),
                                     start=(tt == 0), stop=(tt == ntt - 1),
                                     skip_group_check=True)
                # normalize: recip of ones-row, broadcast via K=1 matmul
                r = smpool.tile([1, TC], F32, tag="r")
                nc.vector.reciprocal(r, ctxps[HD:HD + 1, :])
                bc = psB.tile([64, TC], F32, tag="bc")
                nc.tensor.matmul(bc, _mm(ones_sb[0:1, 0:64]), _mm(r), start=True, stop=True)
                craw = smpool.tile([64, TC], F32, tag="craw")
                nc.vector.tensor_copy(craw, ctxps[0:HD, :])
                nc.vector.tensor_mul(ctx_t[h // 2][bass.ds(base, 64), cs], craw, bc)
            # out-projection for this chunk
            for nt in range(8):
                ops = psO.tile([128, TC], F32, tag="ops")
                for k2 in range(2):
                    nc.tensor.matmul(
                        ops, _mm(wout_sb[:, k2, bass.ds(nt * 128, 128)]),
                        _mm(ctx_t[k2][:, cs]), start=(k2 == 0), stop=(k2 == 1))
                osb = opool.tile([128, TC], F32, tag="osb")
                nc.vector.tensor_scalar_add(osb, ops, bo_sb[:, nt:nt + 1])
                nc.sync.dma_start(out=cc_in[bass.ds(nt * 128, 128), cs], in_=osb)

        # ---- reduce-scatter + final output ----
        nc.gpsimd.collective_compute(
            "ReduceScatter", mybir.AluOpType.add, replica_groups=GROUPS,
            ins=[cc_in[:, :]], outs=[cc_out[:, :]])
        nc.sync.dma_start(out=out_part[:, :], in_=cc_out[:, :])

    nc.finalize()
    return nc


def _rope_tables(offset):
    """cos/sin (32, T), matching the reference's f32 op sequence bit-for-bit."""
    try:
        import jax.numpy as jnp
        inv_freq = 1.0 / (ROPE_BASE ** (jnp.arange(0, HD, 2, dtype=jnp.float32) / HD))
        t = jnp.arange(T, dtype=jnp.float32) + offset
        ang = t[:, None] * inv_freq[None, :]
        cos = np.asarray(jnp.cos(ang)).T.astype(np.float32)
        sin = np.asarray(jnp.sin(ang)).T.astype(np.float32)
    except Exception:
        inv_freq = (1.0 / (ROPE_BASE ** (np.arange(0, HD, 2, dtype=np.float32)
                                         / np.float32(HD)))).astype(np.float32)
        t = (np.arange(T, dtype=np.float32) + np.float32(offset)).astype(np.float32)
        ang = (t[None, :] * inv_freq[:, None]).astype(np.float32)
        cos = np.cos(ang).astype(np.float32)
        sin = np.sin(ang).astype(np.float32)
    return np.ascontiguousarray(cos), np.ascontiguousarray(sin)


def _host_prep(x, Wqkv, bqkv, Wout, bout, offset):
    """Build per-core input maps."""
    # RoPE tables in the permuted row layout [h0e h0o h1e h1o] per 128-row tile.
    # Mirror the reference's exact op sequence (a 1-ulp difference in inv_freq
    # is amplified by t~2048 into ~1e-4 in cos/sin, visible in the k cache), so
    # prefer computing via jnp with the reference's own expressions.
    cos, sin = _rope_tables(float(offset))
    cc = np.tile(cos, (4, 1))                         # (128, T)
    ss = np.concatenate([-sin, sin, -sin, sin], axis=0).astype(np.float32)

    tri = np.triu(np.ones((128, 128), dtype=np.float32))  # valid: tk(p) <= tq(f)

    # local qk column order j -> (head_local, d)
    j = np.arange(DPC)
    r = j % 128
    h_local_qk = (j // 128) * 2 + r // 64
    eo = (r % 64) // 32
    d_qk = 2 * (r % 32) + eo
    h_local_v = j // HD
    d_v = j % HD

    in_maps = []
    for core in range(N_CORES):
        b, g = divmod(core, 4)
        H0 = g * HPC
        qcols = (H0 + h_local_qk) * HD + d_qk
        vcols = (H0 + h_local_v) * HD + d_v
        m = {
            "xT": np.ascontiguousarray(x[b].T),
            "wq": np.ascontiguousarray(Wqkv[:, qcols]),
            "wk": np.ascontiguousarray(Wqkv[:, D + qcols]),
            "wv": np.ascontiguousarray(Wqkv[:, 2 * D + vcols]),
            "bq": np.ascontiguousarray(bqkv[qcols]),
            "bk": np.ascontiguousarray(bqkv[D + qcols]),
            "bv": np.ascontiguousarray(bqkv[2 * D + vcols]),
            "wout": np.ascontiguousarray(Wout[H0 * HD:(H0 + HPC) * HD, :]),
            "bo4": (bout / 4.0).astype(np.float32),
            "cc": cc, "ss": ss, "tri": tri,
        }
        in_maps.append(m)
    return in_maps


def _host_assemble(results):
    out = np.zeros((B, T, D), dtype=np.float32)
    k_cache = np.zeros((B, T, NH, HD), dtype=np.float32)
    v_cache = np.zeros((B, T, NH, HD), dtype=np.float32)

    r = np.arange(DPC)
    h_local = r // HD
    rr = r % HD
    d_nat = 2 * (rr % 32) + rr // 32   # row -> original d (inverse of [e|o] split)

    for core in range(N_CORES):
        b, g = divmod(core, 4)
        res = results[core]
        out[b, :, g * DPC:(g + 1) * DPC] = res["out_part"].T
        kp = res["k_part"]             # (256, T) permuted rows
        k_cache[b, :, g * HPC + h_local, d_nat] = kp
        v_cache[b, :, g * HPC:(g + 1) * HPC, :] = res["v_part"].reshape(T, HPC, HD)
    new_cache = np.stack([k_cache, v_cache], axis=1)
    return out, new_cache


def run_sharded(inputs, trace=False, trace_kwargs=None):
    key = "nc"
    if key not in _BUILD_CACHE:
        _BUILD_CACHE[key] = build_nc()
    nc = _BUILD_CACHE[key]
    in_maps = _host_prep(**inputs)
    kw = {}
    if trace:
        kw["trace"] = True
        if trace_kwargs:
            kw.update(trace_kwargs)
    res = run_bass_kernel_spmd(nc, in_maps, list(range(N_CORES)), **kw)
    return res


def kernel(x, Wqkv, bqkv, Wout, bout, offset):
    res = run_sharded(dict(x=x, Wqkv=Wqkv, bqkv=bqkv, Wout=Wout, bout=bout,
                           offset=offset))
    return _host_assemble(res.results)


# revision 21
# speedup vs baseline: 2.0927x; 2.0927x over previous
"""Trainium2 Bass kernel: causal attention with RoPE (B=2, T=2048, H=16, hd=64).

Sharding: 8 cores = 2 batches x 4 head-groups (4 heads each).
Per core: qkv projection (its 768 weight columns), RoPE, causal attention
(head-parallel), partial out-projection; a per-chunk ReduceScatter over each
4-core group sums the out-projection partials. Host only slices/permutes
inputs and concatenates/permutes outputs.

Layout choices (all transposes are host-side or folded into weights):
  - x is fed transposed: xT (D=1024, T=2048); q,k computed d-major
    (d on partitions), v computed T-major with an appended ones column.
  - Wq/Wk columns host-permuted so each head's q/k rows come out as
    [32 even dims, 32 odd dims] -> RoPE is a 64-row half swap (SBUF->SBUF
    DMA) + 3 elementwise ops against host cos/sin tables.
  - Scores computed transposed (S^T: tk on partitions, tq free) so the
    softmax denominator is produced by the ones column of v_aug during
    the ctx matmul; per-tq reciprocals are broadcast via small matmuls.
  - exp uses a constant shift (exact cancellation in softmax).
  - Matmuls run in float32r (full PE rate, ~1.5e-4 rel err). Matmul inputs
    must come from "rounding" producers, so DMA'd data passes through a DVE
    round-copy and on-chip producers write through f32r-typed views. The
    k/v cache outputs are kept in exact fp32 (separate rounded copies feed
    the attention matmuls).
"""
import sys
import numpy as np

for _p in ("/opt/trn_rl_repo", "/opt/pypackages"):
    if _p not in sys.path:
        sys.path.append(_p)

from contextlib import ExitStack

import concourse.bass as bass
import concourse.tile as tile
from concourse import bacc, mybir
from concourse.bass_utils import run_bass_kernel_spmd

B, T, NH, HD = 2, 2048, 16, 64
D = NH * HD                      # 1024
HPC = 4                          # heads per core
DPC = HPC * HD                   # 256
N_CORES = 8
GROUPS = [[0, 1, 2, 3], [4, 5, 6, 7]]
ROPE_BASE = 10000.0
SHIFT = 12.0                     # constant pre-exp shift; cancels exactly
TC = 512                         # t chunk (moving free dim)
NTC = T // TC                    # 4
F32 = mybir.dt.float32
F32R = mybir.dt.float32r
EXP = mybir.ActivationFunctionType.Exp

_BUILD_CACHE = {}


def _r(ap):
    """f32r view of an fp32 AP (for matmul inputs / rounding writers)."""
    return ap.bitcast(F32R)


def build_nc():
    nc = bacc.Bacc(None, target_bir_lowering=False, num_devices=N_CORES)

    xT = nc.declare_dram_parameter("xT", [D, T], F32, isOutput=False)
    wq = nc.declare_dram_parameter("wq", [D, DPC], F32, isOutput=False)
    wk = nc.declare_dram_parameter("wk", [D, DPC], F32, isOutput=False)
    wv = nc.declare_dram_parameter("wv", [D, DPC], F32, isOutput=False)
    bq = nc.declare_dram_parameter("bq", [DPC], F32, isOutput=False)
    bk = nc.declare_dram_parameter("bk", [DPC], F32, isOutput=False)
    bv = nc.declare_dram_parameter("bv", [DPC], F32, isOutput=False)
    wout = nc.declare_dram_parameter("wout", [DPC, D], F32, isOutput=False)
    bo4 = nc.declare_dram_parameter("bo4", [D], F32, isOutput=False)
    cc = nc.declare_dram_parameter("cc", [128, T], F32, isOutput=False)
    ss = nc.declare_dram_parameter("ss", [128, T], F32, isOutput=False)
    tri = nc.declare_dram_parameter("tri", [128, 128], F32, isOutput=False)

    out_part = nc.declare_dram_parameter("out_part", [NTC, DPC, TC], F32, isOutput=True)
    k_part = nc.declare_dram_parameter("k_part", [DPC, T], F32, isOutput=True)
    v_part = nc.declare_dram_parameter("v_part", [T, DPC], F32, isOutput=True)

    cc_in = nc.dram_tensor("cc_in", [NTC, D, TC], F32)
    cc_out = nc.dram_tensor("cc_out", [NTC, DPC, TC], F32)

    with tile.TileContext(nc) as tc, ExitStack() as ectx:
        ectx.enter_context(nc.allow_low_precision(
            reason="f32r-rounded writes feed f32r matmuls; accumulation stays fp32"))
        consts = ectx.enter_context(tc.tile_pool(name="consts", bufs=1))
        qpool = ectx.enter_context(tc.tile_pool(name="qpool", bufs=1))
        krpool = ectx.enter_context(tc.tile_pool(name="krpool", bufs=1))
        vrpool = ectx.enter_context(tc.tile_pool(name="vrpool", bufs=1))

        # ---- constants into SBUF (DMA raw fp32, then DVE round to f32r) ----
        def _load_rounded(dram_ap, shape, name, raw_pool):
            raw = raw_pool.tile(shape, F32, tag="wraw", name=f"{name}_raw")
            nc.sync.dma_start(out=raw, in_=dram_ap)
            rt = consts.tile(shape, F32R, name=name)
            nc.vector.tensor_copy(out=rt, in_=raw)
            return rt

        p1ctx = ExitStack()
        wrawp = p1ctx.enter_context(tc.tile_pool(name="wrawp", bufs=2))
        wq_sb = _load_rounded(wq.rearrange("(kc kp) m -> kp kc m", kp=128),
                              [128, 8, DPC], "wq_sb", wrawp)
        wk_sb = _load_rounded(wk.rearrange("(kc kp) m -> kp kc m", kp=128),
                              [128, 8, DPC], "wk_sb", wrawp)
        wv_sb = _load_rounded(wv.rearrange("(kc kp) m -> kp kc m", kp=128),
                              [128, 8, DPC], "wv_sb", wrawp)
        wout_sb = _load_rounded(wout.rearrange("(kc kp) m -> kp kc m", kp=128),
                                [128, 2, D], "wout_sb", wrawp)
        bv_r = _load_rounded(bv[None, :], [1, DPC], "bv_r", wrawp)

        bq_sb = consts.tile([128, 2], F32)
        nc.sync.dma_start(out=bq_sb, in_=bq.rearrange("(t p) -> p t", p=128))
        bk_sb = consts.tile([128, 2], F32)
        nc.sync.dma_start(out=bk_sb, in_=bk.rearrange("(t p) -> p t", p=128))
        bo_sb = consts.tile([128, 8], F32)
        nc.sync.dma_start(out=bo_sb, in_=bo4.rearrange("(t p) -> p t", p=128))
        cc_sb = consts.tile([128, T], F32)
        nc.sync.dma_start(out=cc_sb, in_=cc[:, :])
        ss_sb = consts.tile([128, T], F32)
        nc.sync.dma_start(out=ss_sb, in_=ss[:, :])
        tri_sb = consts.tile([128, 128], F32)
        nc.sync.dma_start(out=tri_sb, in_=tri[:, :])
        shift_sb = consts.tile([128, 1], F32)
        nc.vector.memset(shift_sb, -SHIFT)

        ones_f = consts.tile([128, 128], F32)
        nc.vector.memset(ones_f, 1.0)
        ones_r = consts.tile([128, 128], F32R)
        nc.vector.tensor_copy(out=ones_r, in_=ones_f)

        # persistent activations (rounded copies used by attention)
        q_t = [qpool.tile([128, T], F32, tag=f"q{i}", name=f"q{i}") for i in range(2)]
        kr_t = [krpool.tile([128, T], F32R, tag=f"kr{i}", name=f"kr{i}") for i in range(2)]
        # v_aug: (p, tk_tile, head, 65); col 64 is the ones column
        vr_sb = vrpool.tile([128, T // 128, HPC, HD + 1], F32R)

        # ---- phase 1: qkv projection (x streamed, rounded per tile) ----
        xpool = p1ctx.enter_context(tc.tile_pool(name="xpool", bufs=3))
        kexp = p1ctx.enter_context(tc.tile_pool(name="kexp", bufs=1))
        vexp = p1ctx.enter_context(tc.tile_pool(name="vexp", bufs=1))
        swpool = p1ctx.enter_context(tc.tile_pool(name="swpool", bufs=2))
        ps1 = p1ctx.enter_context(tc.tile_pool(name="ps1", bufs=1, space="PSUM"))

        k_t = [kexp.tile([128, T], F32, tag=f"k{i}", name=f"k{i}") for i in range(2)]
        v_ex = vexp.tile([128, T // 128, HPC, HD + 1], F32)
        nc.vector.memset(v_ex[:, :, :, HD:HD + 1], 1.0)

        for tc_ in range(NTC):
            ts = bass.ds(tc_ * TC, TC)
            qps = [ps1.tile([128, TC], F32, tag=f"qps{m}", name=f"qps{m}") for m in range(2)]
            kps = [ps1.tile([128, TC], F32, tag=f"kps{m}", name=f"kps{m}") for m in range(2)]
            vps = [ps1.tile([128, DPC], F32, tag=f"vps{s}", name=f"vps{s}")
                   for s in range(4)]  # one PSUM bank per t-subtile
            for kc in range(8):
                xt = xpool.tile([128, TC], F32, tag="x")
                nc.sync.dma_start(out=xt, in_=xT[bass.ds(kc * 128, 128), ts])
                xtr = xpool.tile([128, TC], F32R, tag="xr")
                nc.vector.tensor_copy(out=xtr, in_=xt)
                for mt in range(2):
                    ms = bass.ds(mt * 128, 128)
                    nc.tensor.matmul(
                        qps[mt], wq_sb[:, kc, ms], xtr,
                        start=(kc == 0), stop=(kc == 7))
                    nc.tensor.matmul(
                        kps[mt], wk_sb[:, kc, ms], xtr,
                        start=(kc == 0), stop=(kc == 7))
                for s in range(4):
                    nc.tensor.matmul(
                        vps[s],
                        xtr[:, bass.ds(s * 128, 128)], wv_sb[:, kc, :],
                        start=(kc == 0), stop=False, skip_group_check=True)
            for s in range(4):  # bias row for v (K=1 matmul of ones x bv)
                nc.tensor.matmul(
                    vps[s],
                    ones_r[0:1, :], bv_r[0:1, :],
                    start=False, stop=True, skip_group_check=True)
            for mt in range(2):
                nc.vector.tensor_scalar_add(_r(q_t[mt][:, ts]), qps[mt], bq_sb[:, mt:mt + 1])
                nc.vector.tensor_scalar_add(k_t[mt][:, ts], kps[mt], bk_sb[:, mt:mt + 1])
            for s in range(4):
                tt = tc_ * 4 + s
                nc.vector.tensor_copy(
                    out=v_ex[:, tt, :, 0:HD],
                    in_=vps[s].rearrange("p (h d) -> p h d", h=HPC))

        # ---- phase 1.5: RoPE on q,k; k/v cache out; rounded copies ----
        for i, tl in enumerate(q_t + k_t):
            is_q = i < 2
            sw = swpool.tile([128, T], F32, tag="sw")
            for h2 in range(2):
                b0 = h2 * 64
                nc.sync.dma_start(out=sw[bass.ds(b0, 32), :], in_=tl[bass.ds(b0 + 32, 32), :])
                nc.sync.dma_start(out=sw[bass.ds(b0 + 32, 32), :], in_=tl[bass.ds(b0, 32), :])
            ro = _r(tl) if is_q else tl
            nc.vector.tensor_mul(ro, tl, cc_sb)
            nc.vector.tensor_mul(sw, sw, ss_sb)
            nc.vector.tensor_add(ro, tl, sw)
        for mt in range(2):
            nc.sync.dma_start(out=k_part[bass.ds(mt * 128, 128), :], in_=k_t[mt])
            nc.vector.tensor_copy(out=kr_t[mt], in_=k_t[mt])
        for h in range(HPC):
            nc.sync.dma_start(
                out=v_part[:, bass.ds(h * HD, HD)].rearrange("(tt p) d -> p tt d", p=128),
                in_=v_ex[:, :, h, 0:HD])
        nc.vector.tensor_copy(out=vr_sb, in_=v_ex)

        p1ctx.close()

        # ---- phase 2: attention ----
        ctxpool = ectx.enter_context(tc.tile_pool(name="ctxpool", bufs=1))
        espool = ectx.enter_context(tc.tile_pool(name="espool", bufs=4))
        smpool = ectx.enter_context(tc.tile_pool(name="smpool", bufs=2))
        opool = ectx.enter_context(tc.tile_pool(name="opool", bufs=3))
        psS = ectx.enter_context(tc.tile_pool(name="psS", bufs=2, space="PSUM"))
        psC = ectx.enter_context(tc.tile_pool(name="psC", bufs=1, space="PSUM"))
        psO = ectx.enter_context(tc.tile_pool(name="psO", bufs=1, space="PSUM"))

        ctx_t = [ctxpool.tile([128, T], F32, tag=f"c{i}", name=f"c{i}") for i in range(2)]

        for c in range(NTC):
            cs = bass.ds(c * TC, TC)
            ctxps = [psC.tile([HD + 1, TC], F32, tag=f"ctxps{h}", name=f"ctxps{h}")
                     for h in range(HPC)]
            for h in range(HPC):
                kt = kr_t[h // 2]
                qt = q_t[h // 2]
                base = (h % 2) * 64
                ntt = 4 * c + 4
                for tt in range(ntt):
                    j = tt - 4 * c
                    sp = psS.tile([128, TC], F32, tag="sps")
                    es = espool.tile([128, TC], F32, tag="es")
                    lhs = kt[bass.ds(base, 64), bass.ds(tt * 128, 128)]
                    if j < 0:  # fully valid tile
                        nc.tensor.matmul(sp, lhs, _r(qt[bass.ds(base, 64), cs]),
                                         start=True, stop=True)
                        nc.scalar.activation(_r(es), sp, EXP,
                                             bias=shift_sb[:, 0:1], scale=0.125)
                    else:  # diagonal-crossing tile
                        # compute at least 256 cols so fp32r stays at 1 cyc/row
                        m0 = min(j * 128, TC - 256)
                        nc.tensor.matmul(
                            sp[:, bass.ds(m0, TC - m0)], lhs,
                            _r(qt[bass.ds(base, 64), bass.ds(c * TC + m0, TC - m0)]),
                            start=True, stop=True)
                        nc.scalar.activation(_r(es[:, bass.ds(j * 128, TC - j * 128)]),
                                             sp[:, bass.ds(j * 128, TC - j * 128)], EXP,
                                             bias=shift_sb[:, 0:1], scale=0.125)
                        nc.vector.tensor_mul(_r(es[:, bass.ds(j * 128, 128)]),
                                             es[:, bass.ds(j * 128, 128)], tri_sb)
                    j0 = max(j, 0) * 128  # columns < j0 are above the diagonal
                    nc.tensor.matmul(ctxps[h][:, bass.ds(j0, TC - j0)],
                                     vr_sb[:, tt, h, :],
                                     _r(es[:, bass.ds(j0, TC - j0)]),
                                     start=(tt == 0), stop=(tt == ntt - 1),
                                     skip_group_check=True)
            # normalize: fast reciprocal of the ones-row, then multiply with a
            # stride-0 partition-broadcast operand
            for h in range(HPC):
                dsb = smpool.tile([1, TC], F32, tag="dsb")
                nc.vector.tensor_copy(out=dsb, in_=ctxps[h][HD:HD + 1, :])
                r = smpool.tile([1, TC], F32, tag="r")
                nc.vector.reciprocal_approx_fast(out=r, in_=dsb)
                rb = smpool.tile([64, TC], F32, tag="rb")
                nc.gpsimd.partition_broadcast(out_ap=rb, in_ap=r[0:1, :])
                nc.vector.tensor_mul(
                    _r(ctx_t[h // 2][bass.ds((h % 2) * 64, 64), cs]),
                    ctxps[h][0:HD, :], rb)
            # out-projection for this chunk
            for nt in range(8):
                ops = psO.tile([128, TC], F32, tag="ops")
                for k2 in range(2):
                    nc.tensor.matmul(
                        ops, wout_sb[:, k2, bass.ds(nt * 128, 128)],
                        _r(ctx_t[k2][:, cs]), start=(k2 == 0), stop=(k2 == 1))
                osb = opool.tile([128, TC], F32, tag="osb")
                nc.vector.tensor_scalar_add(osb, ops, bo_sb[:, nt:nt + 1])
                nc.sync.dma_start(out=cc_in[c, bass.ds(nt * 128, 128), :], in_=osb)
            # reduce-scatter this chunk's partials while later chunks compute
            nc.gpsimd.collective_compute(
                "ReduceScatter", mybir.AluOpType.add, replica_groups=GROUPS,
                ins=[cc_in[c]], outs=[cc_out[c]])
            nc.sync.dma_start(out=out_part[c], in_=cc_out[c])

    nc.finalize()
    return nc


def _rope_tables(offset):
    """cos/sin (32, T), matching the reference's f32 op sequence bit-for-bit."""
    try:
        import jax.numpy as jnp
        inv_freq = 1.0 / (ROPE_BASE ** (jnp.arange(0, HD, 2, dtype=jnp.float32) / HD))
        t = jnp.arange(T, dtype=jnp.float32) + offset
        ang = t[:, None] * inv_freq[None, :]
        cos = np.asarray(jnp.cos(ang)).T.astype(np.float32)
        sin = np.asarray(jnp.sin(ang)).T.astype(np.float32)
    except Exception:
        inv_freq = (1.0 / (ROPE_BASE ** (np.arange(0, HD, 2, dtype=np.float32)
                                         / np.float32(HD)))).astype(np.float32)
        t = (np.arange(T, dtype=np.float32) + np.float32(offset)).astype(np.float32)
        ang = (t[None, :] * inv_freq[:, None]).astype(np.float32)
        cos = np.cos(ang).astype(np.float32)
        sin = np.sin(ang).astype(np.float32)
    return np.ascontiguousarray(cos), np.ascontiguousarray(sin)


def _host_prep(x, Wqkv, bqkv, Wout, bout, offset):
    """Build per-core input maps."""
    cos, sin = _rope_tables(float(offset))
    cc = np.tile(cos, (4, 1))                         # (128, T)
    ss = np.concatenate([-sin, sin, -sin, sin], axis=0).astype(np.float32)

    tri = np.triu(np.ones((128, 128), dtype=np.float32))  # valid: tk(p) <= tq(f)

    # local qk column order j -> (head_local, d)
    j = np.arange(DPC)
    r = j % 128
    h_local_qk = (j // 128) * 2 + r // 64
    eo = (r % 64) // 32
    d_qk = 2 * (r % 32) + eo
    h_local_v = j // HD
    d_v = j % HD

    in_maps = []
    for core in range(N_CORES):
        b, g = divmod(core, 4)
        H0 = g * HPC
        qcols = (H0 + h_local_qk) * HD + d_qk
        vcols = (H0 + h_local_v) * HD + d_v
        m = {
            "xT": np.ascontiguousarray(x[b].T),
            "wq": np.ascontiguousarray(Wqkv[:, qcols]),
            "wk": np.ascontiguousarray(Wqkv[:, D + qcols]),
            "wv": np.ascontiguousarray(Wqkv[:, 2 * D + vcols]),
            "bq": np.ascontiguousarray(bqkv[qcols]),
            "bk": np.ascontiguousarray(bqkv[D + qcols]),
            "bv": np.ascontiguousarray(bqkv[2 * D + vcols]),
            "wout": np.ascontiguousarray(Wout[H0 * HD:(H0 + HPC) * HD, :]),
            "bo4": (bout / 4.0).astype(np.float32),
            "cc": cc, "ss": ss, "tri": tri,
        }
        in_maps.append(m)
    return in_maps


def _host_assemble(results):
    out = np.zeros((B, T, D), dtype=np.float32)
    k_cache = np.zeros((B, T, NH, HD), dtype=np.float32)
    v_cache = np.zeros((B, T, NH, HD), dtype=np.float32)

    r = np.arange(DPC)
    h_local = r // HD
    rr = r % HD
    d_nat = 2 * (rr % 32) + rr // 32   # row -> original d (inverse of [e|o] split)

    for core in range(N_CORES):
        b, g = divmod(core, 4)
        res = results[core]
        op = res["out_part"]           # (NTC, 256, TC) chunk-major out^T slices
        out[b, :, g * DPC:(g + 1) * DPC] = op.transpose(0, 2, 1).reshape(T, DPC)
        kp = res["k_part"]             # (256, T) permuted rows
        k_cache[b, :, g * HPC + h_local, d_nat] = kp
        v_cache[b, :, g * HPC:(g + 1) * HPC, :] = res["v_part"].reshape(T, HPC, HD)
    new_cache = np.stack([k_cache, v_cache], axis=1)
    return out, new_cache


def run_sharded(inputs, trace=False, trace_kwargs=None):
    key = "nc"
    if key not in _BUILD_CACHE:
        _BUILD_CACHE[key] = build_nc()
    nc = _BUILD_CACHE[key]
    in_maps = _host_prep(**inputs)
    kw = {}
    if trace:
        kw["trace"] = True
        if trace_kwargs:
            kw.update(trace_kwargs)
    # The axon-tunneled device intermittently reports NRT_EXEC_UNIT_UNRECOVERABLE
    # on the first attempt after a prior session; a retry clears it.
    last = None
    for attempt in range(3):
        try:
            return run_bass_kernel_spmd(nc, in_maps, list(range(N_CORES)), **kw)
        except Exception as e:  # noqa: BLE001
            last = e
            import time as _time
            _time.sleep(10 * (attempt + 1))
    raise last


def kernel(x, Wqkv, bqkv, Wout, bout, offset):
    res = run_sharded(dict(x=x, Wqkv=Wqkv, bqkv=bqkv, Wout=Wout, bout=bout,
                           offset=offset))
    return _host_assemble(res.results)
